# revision 6
# baseline (speedup 1.0000x reference)
"""AttnBlock (GroupNorm + single-head spatial self-attention + residual) on
8 Trainium2 NeuronCores.

Sharding: batch (4) x query-half (2) -> 8 independent shards, one per core.
Every core runs the SAME program on different data: the host rolls the
flattened spatial axis by 2048 for odd cores so each core's queries are the
first 2048 columns of its local x, while K/V/GroupNorm see the full 4096.

Per-core pipeline (all on device):
  1. GroupNorm stats: bn_stats/bn_aggr per channel, then two tiny fp32
     matmuls reduce across partitions (group stats) and broadcast back.
  2. GN affine (alpha, beta) folded into the Q/K/V weights and biases.
  3. Q/K 1x1 convs -> [c, n] layout; V conv emitted transposed [n, c]
     directly by swapping matmul operands.
  4. Attention with transposed scores: ST[j, i] = k^T q, P = exp(ST/16)
     (softmax max-subtraction skipped; scores are O(10) so exp is safe),
     attn[c, i] = sum_j vT[j, c] P[j, i] accumulated over j in PSUM.
     Softmax denominator Z via a zero-padded ones-column matmul; 1/Z is
     broadcast across partitions with a row-0-ones matmul.
  5. Proj conv + bias + residual, DMA out.

Heavy matmuls run in float32r (full PE rate, ~1.5e-4 rel err); tiny
GroupNorm matmuls in float32.
"""
import numpy as np

B, C, H, W = 4, 256, 64, 64
N = H * W            # 4096 spatial positions
NQ = N // 2          # 2048 queries per core
P = 128              # partitions
CT = C // P          # 2 channel tiles
NUM_GROUPS = 8
EPS = 1e-5
SCALE = float(C) ** -0.5

_CACHED = {}


def _build():
    import concourse.bass as bass
    import concourse.mybir as mybir
    import concourse.tile as tile
    from concourse import bacc

    dt = mybir.dt
    AF = mybir.ActivationFunctionType
    Alu = mybir.AluOpType

    nc = bacc.Bacc("TRN2", debug=False, num_devices=8)

    x_d = nc.dram_tensor("x", [C, N], dt.float32r, kind="ExternalInput")
    wq_d = nc.dram_tensor("wqT", [C, C], dt.float32, kind="ExternalInput")
    wk_d = nc.dram_tensor("wkT", [C, C], dt.float32, kind="ExternalInput")
    wv_d = nc.dram_tensor("wvT", [C, C], dt.float32, kind="ExternalInput")
    wp_d = nc.dram_tensor("wpT", [C, C], dt.float32, kind="ExternalInput")
    bq_d = nc.dram_tensor("bq", [C], dt.float32, kind="ExternalInput")
    bk_d = nc.dram_tensor("bk", [C], dt.float32, kind="ExternalInput")
    bv_d = nc.dram_tensor("bv", [C], dt.float32, kind="ExternalInput")
    bp_d = nc.dram_tensor("bp", [C], dt.float32, kind="ExternalInput")
    gsc_d = nc.dram_tensor("gn_scale", [C], dt.float32, kind="ExternalInput")
    gbi_d = nc.dram_tensor("gn_bias", [C], dt.float32, kind="ExternalInput")
    sel_d = nc.dram_tensor("sel", [P, 4], dt.float32, kind="ExternalInput")
    e4_d = nc.dram_tensor("E4", [4, P], dt.float32, kind="ExternalInput")
    e0_d = nc.dram_tensor("e0_ones", [P, P], dt.float32r, kind="ExternalInput")
    out_d = nc.dram_tensor("out", [C, NQ], dt.float32, kind="ExternalOutput")

    x_ap = x_d.ap().rearrange("(t p) n -> p t n", p=P)
    out_ap = out_d.ap().rearrange("(t p) n -> p t n", p=P)

    with tile.TileContext(nc) as tc:
        with (
            nc.allow_low_precision(reason="float32r rounding is intentional"),
            tc.tile_pool(name="persist", bufs=1) as pe_,
            tc.tile_pool(name="pt", bufs=5) as ptp,
            tc.tile_pool(name="tmp", bufs=3) as tmp,
            tc.tile_pool(name="mm", bufs=2, space="PSUM") as mmp,
            tc.tile_pool(name="acc", bufs=4, space="PSUM") as accp,
            tc.tile_pool(name="zp", bufs=2, space="PSUM") as zpp,
        ):
            # ---------- load persistent data ----------
            x_r = pe_.tile([P, CT, N], dt.float32r, tag="x")
            stats = pe_.tile([P, CT, 8, 6], dt.float32, tag="stats")
            for ck in range(8):
                s = slice(ck * 512, (ck + 1) * 512)
                nc.sync.dma_start(x_r[:, :, s], x_ap[:, :, s])
                for t in range(CT):
                    nc.vector.bn_stats(stats[:, t, ck, :], x_r[:, t, s])

            wT = {}
            for nm, d in (("q", wq_d), ("k", wk_d), ("v", wv_d), ("p", wp_d)):
                wT[nm] = pe_.tile([P, CT, C], dt.float32, tag=f"w{nm}", name=f"w{nm}")
                nc.sync.dma_start(wT[nm], d.ap().rearrange("(t p) o -> p t o", p=P))
            bvec = {}
            for nm, d in (("q", bq_d), ("k", bk_d), ("v", bv_d), ("p", bp_d),
                          ("gsc", gsc_d), ("gbi", gbi_d)):
                bvec[nm] = pe_.tile([P, CT], dt.float32, tag=f"b{nm}", name=f"b{nm}")
                nc.sync.dma_start(bvec[nm], d.ap().rearrange("(t p) -> p t", p=P))
            sel_sb = pe_.tile([P, 4], dt.float32, tag="sel")
            nc.sync.dma_start(sel_sb, sel_d.ap())
            e4_sb = pe_.tile([4, P], dt.float32, tag="e4")
            nc.sync.dma_start(e4_sb, e4_d.ap())
            e0_sb = pe_.tile([P, P], dt.float32r, tag="e0")
            nc.sync.dma_start(e0_sb, e0_d.ap())
            # ---------- GroupNorm statistics ----------
            mv = pe_.tile([P, CT, 2], dt.float32, tag="mv")
            for t in range(CT):
                nc.vector.bn_aggr(mv[:, t, :], stats[:, t])
            # stats_cat cols: mean_t0, mean_t1, meansq_t0, meansq_t1
            scat = pe_.tile([P, 4], dt.float32, tag="scat")
            for t in range(CT):
                nc.vector.tensor_copy(scat[:, t : t + 1], mv[:, t, 0:1])
                sq = tmp.tile([P, 1], dt.float32, tag="sq")
                nc.vector.tensor_mul(sq, mv[:, t, 0:1], mv[:, t, 0:1])
                nc.vector.tensor_add(scat[:, 2 + t : 3 + t], sq, mv[:, t, 1:2])
            gs_ps = mmp.tile([4, 4], dt.float32, tag="mm")
            nc.tensor.matmul(gs_ps, sel_sb, scat, start=True, stop=True)
            gs = pe_.tile([4, 4], dt.float32, tag="gs")
            nc.vector.tensor_copy(gs, gs_ps)
            # var = meansq - mean^2 ; rstd = rsqrt(var + eps) + one Newton step
            msq = pe_.tile([4, 2], dt.float32, tag="msq")
            nc.vector.tensor_mul(msq, gs[:, 0:2], gs[:, 0:2])
            veps = pe_.tile([4, 2], dt.float32, tag="veps")
            nc.vector.tensor_sub(veps, gs[:, 2:4], msq)
            nc.vector.tensor_scalar_add(veps, veps, EPS)
            sqv = pe_.tile([4, 2], dt.float32, tag="sqv")
            nc.scalar.activation(sqv, veps, AF.Sqrt)
            y0 = pe_.tile([4, 2], dt.float32, tag="y0")
            nc.vector.reciprocal(y0, sqv)
            yy = pe_.tile([4, 2], dt.float32, tag="yy")
            nc.vector.tensor_mul(yy, y0, y0)
            nc.vector.tensor_mul(yy, veps, yy)
            nc.vector.tensor_scalar(yy, yy, -0.5, 1.5, Alu.mult, Alu.add)
            mr = pe_.tile([4, 4], dt.float32, tag="mr")
            nc.vector.tensor_copy(mr[:, 0:2], gs[:, 0:2])
            nc.vector.tensor_mul(mr[:, 2:4], y0, yy)
            bc_ps = mmp.tile([P, 4], dt.float32, tag="mm")
            nc.tensor.matmul(bc_ps, e4_sb, mr, start=True, stop=True)
            bc = pe_.tile([P, 4], dt.float32, tag="bc")
            nc.vector.tensor_copy(bc, bc_ps)
            alpha = pe_.tile([P, CT], dt.float32, tag="alpha")
            nc.vector.tensor_mul(alpha, bc[:, 2:4], bvec["gsc"])
            beta = pe_.tile([P, CT], dt.float32, tag="beta")
            nc.vector.tensor_mul(beta, bc[:, 0:2], alpha)
            nc.vector.tensor_sub(beta, bvec["gbi"], beta)

            # ---------- fold GN affine into weights & biases ----------
            wsc = {}
            for nm in ("q", "k", "v"):
                wsc[nm] = pe_.tile([P, CT, C], dt.float32r, tag=f"wsc{nm}", name=f"wsc{nm}")
                for t in range(CT):
                    nc.vector.tensor_scalar_mul(
                        wsc[nm][:, t], wT[nm][:, t], alpha[:, t : t + 1]
                    )
            wp_r = pe_.tile([P, CT, C], dt.float32r, tag="wscp")
            nc.vector.tensor_copy(wp_r, wT["p"])

            bfold = {}
            for nm in ("q", "k", "v"):
                bfold[nm] = pe_.tile([P, CT], dt.float32, tag=f"bf{nm}", name=f"bf{nm}")
                for h in range(CT):
                    bb_ps = mmp.tile([P, 1], dt.float32, tag="mm")
                    for t in range(CT):
                        nc.tensor.matmul(
                            bb_ps,
                            wT[nm][:, t, h * P : (h + 1) * P],
                            beta[:, t : t + 1],
                            start=(t == 0),
                            stop=(t == CT - 1),
                        )
                    nc.vector.tensor_add(
                        bfold[nm][:, h : h + 1], bb_ps, bvec[nm][:, h : h + 1]
                    )

            # ---------- Q/K/V 1x1 convs ----------
            k_sb = pe_.tile([P, CT, N], dt.float32r, tag="k")
            q_sb = pe_.tile([P, CT, NQ], dt.float32r, tag="q")
            for h in range(CT):
                for ck in range(8):
                    s = slice(ck * 512, (ck + 1) * 512)
                    cp = mmp.tile([P, 512], dt.float32, tag="mm")
                    for t in range(CT):
                        nc.tensor.matmul(
                            cp,
                            wsc["k"][:, t, h * P : (h + 1) * P],
                            x_r[:, t, s],
                            start=(t == 0),
                            stop=(t == CT - 1),
                        )
                    nc.scalar.activation(
                        k_sb[:, h, s], cp, AF.Identity,
                        bias=bfold["k"][:, h : h + 1], scale=1.0,
                    )
            for h in range(CT):
                for ck in range(4):
                    s = slice(ck * 512, (ck + 1) * 512)
                    cp = mmp.tile([P, 512], dt.float32, tag="mm")
                    for t in range(CT):
                        nc.tensor.matmul(
                            cp,
                            wsc["q"][:, t, h * P : (h + 1) * P],
                            x_r[:, t, s],
                            start=(t == 0),
                            stop=(t == CT - 1),
                        )
                    nc.scalar.activation(
                        q_sb[:, h, s], cp, AF.Identity,
                        bias=bfold["q"][:, h : h + 1], scale=1.0,
                    )
            # vT[n, c] (v bias is applied after attention: softmax rows sum
            # to 1, so attn(v + b) = attn(v) + b)
            vT = pe_.tile([P, 32, C], dt.float32r, tag="vT")
            for jt in range(32):
                vp = mmp.tile([P, C], dt.float32, tag="mm")
                for t in range(CT):
                    nc.tensor.matmul(
                        vp,
                        x_r[:, t, jt * P : (jt + 1) * P],
                        wsc["v"][:, t, :],
                        start=(t == 0),
                        stop=(t == CT - 1),
                    )
                nc.vector.tensor_copy(vT[:, jt], vp)

            # ---------- attention + proj, per 512-wide query chunk ----------
            # The finalize (softmax normalization) and proj for chunk ic-1
            # are emitted after chunk ic's j-loop so their cross-engine
            # latency hides under the next chunk's matmul stream.
            attn = pe_.tile([P, CT, NQ], dt.float32r, tag="attn")
            NIC = NQ // 512
            pend = {}

            def finalize(ic):
                isl, a_ps, z_ps = pend.pop(ic)
                # 1/Z: reciprocal of psum row 0, broadcast on idle GpSimd
                zr = tmp.tile([1, 512], dt.float32, tag="zr", name=f"zr{ic}")
                nc.vector.reciprocal(zr, z_ps[0:1, :])
                zb = tmp.tile([P, 512], dt.float32, tag="zb", name=f"zb{ic}")
                nc.gpsimd.partition_broadcast(zb, zr)
                for ch in range(CT):
                    nc.vector.tensor_mul(attn[:, ch, isl], a_ps[ch], zb)
                    nc.vector.tensor_scalar_add(
                        attn[:, ch, isl], attn[:, ch, isl],
                        bfold["v"][:, ch : ch + 1],
                    )
                o_sb = tmp.tile([P, CT, 512], dt.float32, tag="o", name=f"o{ic}")
                for h in range(CT):
                    pp = zpp.tile([P, 512], dt.float32, tag="z", name=f"pp{ic}_{h}")
                    for t in range(CT):
                        nc.tensor.matmul(
                            pp,
                            wp_r[:, t, h * P : (h + 1) * P],
                            attn[:, t, isl],
                            start=(t == 0),
                            stop=(t == CT - 1),
                        )
                    nc.vector.tensor_scalar_add(
                        o_sb[:, h], pp, bvec["p"][:, h : h + 1]
                    )
                    nc.vector.tensor_add(o_sb[:, h], o_sb[:, h], x_r[:, h, isl])
                nc.sync.dma_start(out_ap[:, :, isl], o_sb)

            for ic in range(NIC):
                isl = slice(ic * 512, (ic + 1) * 512)
                a_ps = [accp.tile([P, 512], dt.float32, tag="acc", name=f"acc{ic}_{i}") for i in range(CT)]
                z_ps = zpp.tile([P, 512], dt.float32, tag="z")
                for jt in range(32):
                    st = mmp.tile([P, 512], dt.float32, tag="mm")
                    for h in range(CT):
                        nc.tensor.matmul(
                            st,
                            k_sb[:, h, jt * P : (jt + 1) * P],
                            q_sb[:, h, isl],
                            start=(h == 0),
                            stop=(h == CT - 1),
                        )
                    pt = ptp.tile([P, 512], dt.float32r, tag="pt")
                    nc.scalar.activation(pt, st, AF.Exp, scale=SCALE)
                    for ch in range(CT):
                        nc.tensor.matmul(
                            a_ps[ch],
                            vT[:, jt, ch * P : (ch + 1) * P],
                            pt,
                            start=(jt == 0),
                            stop=(jt == 31),
                        )
                    nc.tensor.matmul(
                        z_ps, e0_sb, pt, start=(jt == 0), stop=(jt == 31)
                    )
                pend[ic] = (isl, a_ps, z_ps)
                if ic > 0:
                    finalize(ic - 1)
            finalize(NIC - 1)

    nc.compile()
    return nc


def _get_nc():
    if "nc" not in _CACHED:
        _CACHED["nc"] = _build()
    return _CACHED["nc"]


def _host_constants():
    sel = np.zeros((P, 4), np.float32)
    e4 = np.zeros((4, P), np.float32)
    for g in range(4):
        sel[g * 32 : (g + 1) * 32, g] = 1.0 / 32.0
        e4[g, g * 32 : (g + 1) * 32] = 1.0
    e0 = np.zeros((P, P), np.float32)
    e0[:, 0] = 1.0  # lhsT col 0 = ones -> psum row 0 = column sums
    return sel, e4, e0


def kernel(x, gn_scale, gn_bias, wq, bq, wk, bk, wv, bv, wp, bp, _trace=False):
    from concourse.bass_utils import run_bass_kernel_spmd

    nc = _get_nc()
    x = np.ascontiguousarray(np.asarray(x, np.float32)).reshape(B, C, N)
    sel, e4, e0 = _host_constants()
    shared = {
        "wqT": np.ascontiguousarray(np.asarray(wq, np.float32).T),
        "wkT": np.ascontiguousarray(np.asarray(wk, np.float32).T),
        "wvT": np.ascontiguousarray(np.asarray(wv, np.float32).T),
        "wpT": np.ascontiguousarray(np.asarray(wp, np.float32).T),
        "bq": np.asarray(bq, np.float32),
        "bk": np.asarray(bk, np.float32),
        "bv": np.asarray(bv, np.float32),
        "bp": np.asarray(bp, np.float32),
        "gn_scale": np.asarray(gn_scale, np.float32),
        "gn_bias": np.asarray(gn_bias, np.float32),
        "sel": sel, "E4": e4, "e0_ones": e0,
    }
    in_maps = []
    for core in range(8):
        b, qh = core // 2, core % 2
        xl = x[b] if qh == 0 else np.ascontiguousarray(
            np.concatenate([x[b][:, NQ:], x[b][:, :NQ]], axis=1)
        )
        in_maps.append({**shared, "x": xl})

    res = run_bass_kernel_spmd(nc, in_maps, core_ids=list(range(8)), trace=_trace)
    out = np.empty((B, C, N), np.float32)
    for core in range(8):
        b, qh = core // 2, core % 2
        out[b][:, qh * NQ : (qh + 1) * NQ] = res.results[core]["out"]
    if _trace:
        _CACHED["last_results"] = res
    return out.reshape(B, C, H, W)


# revision 7
# speedup vs baseline: 1.1454x; 1.1454x over previous
"""AttnBlock (GroupNorm + single-head spatial self-attention + residual) on
8 Trainium2 NeuronCores.

Sharding: batch (4) x query-half (2) -> 8 independent shards, one per core.
Every core runs the SAME program on different data: the host rolls the
flattened spatial axis by 2048 for odd cores so each core's queries are the
first 2048 columns of its local x, while K/V/GroupNorm see the full 4096.

Per-core pipeline (all on device):
  1. GroupNorm stats: bn_stats/bn_aggr per channel, then two tiny fp32
     matmuls reduce across partitions (group stats) and broadcast back.
  2. GN affine (alpha, beta) folded into the Q/K/V weights and biases.
  3. Q/K 1x1 convs -> [c, n] layout; V conv emitted transposed [n, c]
     directly by swapping matmul operands.
  4. Attention with transposed scores: ST[j, i] = k^T q, P = exp(ST/16)
     (softmax max-subtraction skipped; scores are O(10) so exp is safe),
     attn[c, i] = sum_j vT[j, c] P[j, i] accumulated over j in PSUM.
     Softmax denominator Z via a zero-padded ones-column matmul; 1/Z is
     broadcast across partitions with a row-0-ones matmul.
  5. Proj conv + bias + residual, DMA out.

Heavy matmuls run in float32r (full PE rate, ~1.5e-4 rel err); tiny
GroupNorm matmuls in float32.
"""
import numpy as np

B, C, H, W = 4, 256, 64, 64
N = H * W            # 4096 spatial positions
NQ = N // 2          # 2048 queries per core
P = 128              # partitions
CT = C // P          # 2 channel tiles
NUM_GROUPS = 8
EPS = 1e-5
SCALE = float(C) ** -0.5

_CACHED = {}


def _build():
    import concourse.bass as bass
    import concourse.mybir as mybir
    import concourse.tile as tile
    from concourse import bacc

    dt = mybir.dt
    AF = mybir.ActivationFunctionType
    Alu = mybir.AluOpType

    nc = bacc.Bacc("TRN2", debug=False, num_devices=8)

    x_d = nc.dram_tensor("x", [C, N], dt.float32r, kind="ExternalInput")
    wq_d = nc.dram_tensor("wqT", [C, C], dt.float32, kind="ExternalInput")
    wk_d = nc.dram_tensor("wkT", [C, C], dt.float32, kind="ExternalInput")
    wv_d = nc.dram_tensor("wvT", [C, C], dt.float32, kind="ExternalInput")
    wp_d = nc.dram_tensor("wpT", [C, C], dt.float32, kind="ExternalInput")
    bq_d = nc.dram_tensor("bq", [C], dt.float32, kind="ExternalInput")
    bk_d = nc.dram_tensor("bk", [C], dt.float32, kind="ExternalInput")
    bv_d = nc.dram_tensor("bv", [C], dt.float32, kind="ExternalInput")
    bp_d = nc.dram_tensor("bp", [C], dt.float32, kind="ExternalInput")
    gsc_d = nc.dram_tensor("gn_scale", [C], dt.float32, kind="ExternalInput")
    gbi_d = nc.dram_tensor("gn_bias", [C], dt.float32, kind="ExternalInput")
    sel_d = nc.dram_tensor("sel", [P, 4], dt.float32, kind="ExternalInput")
    e4_d = nc.dram_tensor("E4", [4, P], dt.float32, kind="ExternalInput")
    e0_d = nc.dram_tensor("e0_ones", [P, P], dt.float32r, kind="ExternalInput")
    out_d = nc.dram_tensor("out", [C, NQ], dt.float32, kind="ExternalOutput")

    x_ap = x_d.ap().rearrange("(t p) n -> p t n", p=P)
    out_ap = out_d.ap().rearrange("(t p) n -> p t n", p=P)

    with tile.TileContext(nc) as tc:
        with (
            nc.allow_low_precision(reason="float32r rounding is intentional"),
            tc.tile_pool(name="persist", bufs=1) as pe_,
            tc.tile_pool(name="pt", bufs=5) as ptp,
            tc.tile_pool(name="tmp", bufs=3) as tmp,
            tc.tile_pool(name="mm", bufs=3, space="PSUM") as mmp,
            tc.tile_pool(name="acc", bufs=4, space="PSUM") as accp,
            tc.tile_pool(name="zp", bufs=1, space="PSUM") as zpp,
        ):
            # ---------- load persistent data ----------
            x_r = pe_.tile([P, CT, N], dt.float32r, tag="x")
            stats = pe_.tile([P, CT, 8, 6], dt.float32, tag="stats")
            for ck in range(8):
                s = slice(ck * 512, (ck + 1) * 512)
                nc.sync.dma_start(x_r[:, :, s], x_ap[:, :, s])
                for t in range(CT):
                    nc.vector.bn_stats(stats[:, t, ck, :], x_r[:, t, s])

            wT = {}
            for nm, d in (("q", wq_d), ("k", wk_d), ("v", wv_d), ("p", wp_d)):
                wT[nm] = pe_.tile([P, CT, C], dt.float32, tag=f"w{nm}", name=f"w{nm}")
                nc.sync.dma_start(wT[nm], d.ap().rearrange("(t p) o -> p t o", p=P))
            bvec = {}
            for nm, d in (("q", bq_d), ("k", bk_d), ("v", bv_d), ("p", bp_d),
                          ("gsc", gsc_d), ("gbi", gbi_d)):
                bvec[nm] = pe_.tile([P, CT], dt.float32, tag=f"b{nm}", name=f"b{nm}")
                nc.sync.dma_start(bvec[nm], d.ap().rearrange("(t p) -> p t", p=P))
            sel_sb = pe_.tile([P, 4], dt.float32, tag="sel")
            nc.sync.dma_start(sel_sb, sel_d.ap())
            e4_sb = pe_.tile([4, P], dt.float32, tag="e4")
            nc.sync.dma_start(e4_sb, e4_d.ap())
            e0_sb = pe_.tile([P, P], dt.float32r, tag="e0")
            nc.sync.dma_start(e0_sb, e0_d.ap())
            # ---------- GroupNorm statistics ----------
            mv = pe_.tile([P, CT, 2], dt.float32, tag="mv")
            for t in range(CT):
                nc.vector.bn_aggr(mv[:, t, :], stats[:, t])
            # stats_cat cols: mean_t0, mean_t1, meansq_t0, meansq_t1
            scat = pe_.tile([P, 4], dt.float32, tag="scat")
            for t in range(CT):
                nc.vector.tensor_copy(scat[:, t : t + 1], mv[:, t, 0:1])
                sq = tmp.tile([P, 1], dt.float32, tag="sq")
                nc.vector.tensor_mul(sq, mv[:, t, 0:1], mv[:, t, 0:1])
                nc.vector.tensor_add(scat[:, 2 + t : 3 + t], sq, mv[:, t, 1:2])
            gs_ps = mmp.tile([4, 4], dt.float32, tag="mm")
            nc.tensor.matmul(gs_ps, sel_sb, scat, start=True, stop=True)
            gs = pe_.tile([4, 4], dt.float32, tag="gs")
            nc.vector.tensor_copy(gs, gs_ps)
            # var = meansq - mean^2 ; rstd = rsqrt(var + eps) + one Newton step
            msq = pe_.tile([4, 2], dt.float32, tag="msq")
            nc.vector.tensor_mul(msq, gs[:, 0:2], gs[:, 0:2])
            veps = pe_.tile([4, 2], dt.float32, tag="veps")
            nc.vector.tensor_sub(veps, gs[:, 2:4], msq)
            nc.vector.tensor_scalar_add(veps, veps, EPS)
            sqv = pe_.tile([4, 2], dt.float32, tag="sqv")
            nc.scalar.activation(sqv, veps, AF.Sqrt)
            y0 = pe_.tile([4, 2], dt.float32, tag="y0")
            nc.vector.reciprocal(y0, sqv)
            yy = pe_.tile([4, 2], dt.float32, tag="yy")
            nc.vector.tensor_mul(yy, y0, y0)
            nc.vector.tensor_mul(yy, veps, yy)
            nc.vector.tensor_scalar(yy, yy, -0.5, 1.5, Alu.mult, Alu.add)
            mr = pe_.tile([4, 4], dt.float32, tag="mr")
            nc.vector.tensor_copy(mr[:, 0:2], gs[:, 0:2])
            nc.vector.tensor_mul(mr[:, 2:4], y0, yy)
            bc_ps = mmp.tile([P, 4], dt.float32, tag="mm")
            nc.tensor.matmul(bc_ps, e4_sb, mr, start=True, stop=True)
            bc = pe_.tile([P, 4], dt.float32, tag="bc")
            nc.vector.tensor_copy(bc, bc_ps)
            alpha = pe_.tile([P, CT], dt.float32, tag="alpha")
            nc.vector.tensor_mul(alpha, bc[:, 2:4], bvec["gsc"])
            beta = pe_.tile([P, CT], dt.float32, tag="beta")
            nc.vector.tensor_mul(beta, bc[:, 0:2], alpha)
            nc.vector.tensor_sub(beta, bvec["gbi"], beta)

            # ---------- fold GN affine into weights & biases ----------
            wsc = {}
            for nm in ("q", "k", "v"):
                wsc[nm] = pe_.tile([P, CT, C], dt.float32r, tag=f"wsc{nm}", name=f"wsc{nm}")
                for t in range(CT):
                    nc.vector.tensor_scalar_mul(
                        wsc[nm][:, t], wT[nm][:, t], alpha[:, t : t + 1]
                    )
            wp_r = pe_.tile([P, CT, C], dt.float32r, tag="wscp")
            nc.vector.tensor_copy(wp_r, wT["p"])

            bfold = {}
            for nm in ("q", "k", "v"):
                bfold[nm] = pe_.tile([P, CT], dt.float32, tag=f"bf{nm}", name=f"bf{nm}")
                for h in range(CT):
                    bb_ps = mmp.tile([P, 1], dt.float32, tag="mm")
                    for t in range(CT):
                        nc.tensor.matmul(
                            bb_ps,
                            wT[nm][:, t, h * P : (h + 1) * P],
                            beta[:, t : t + 1],
                            start=(t == 0),
                            stop=(t == CT - 1),
                        )
                    nc.vector.tensor_add(
                        bfold[nm][:, h : h + 1], bb_ps, bvec[nm][:, h : h + 1]
                    )

            # ---------- Q/K/V 1x1 convs ----------
            k_sb = pe_.tile([P, CT, N], dt.float32r, tag="k")
            q_sb = pe_.tile([P, CT, NQ], dt.float32r, tag="q")
            for h in range(CT):
                for ck in range(8):
                    s = slice(ck * 512, (ck + 1) * 512)
                    cp = mmp.tile([P, 512], dt.float32, tag="mm")
                    for t in range(CT):
                        nc.tensor.matmul(
                            cp,
                            wsc["k"][:, t, h * P : (h + 1) * P],
                            x_r[:, t, s],
                            start=(t == 0),
                            stop=(t == CT - 1),
                        )
                    nc.scalar.activation(
                        k_sb[:, h, s], cp, AF.Identity,
                        bias=bfold["k"][:, h : h + 1], scale=1.0,
                    )
            for h in range(CT):
                for ck in range(4):
                    s = slice(ck * 512, (ck + 1) * 512)
                    cp = mmp.tile([P, 512], dt.float32, tag="mm")
                    for t in range(CT):
                        nc.tensor.matmul(
                            cp,
                            wsc["q"][:, t, h * P : (h + 1) * P],
                            x_r[:, t, s],
                            start=(t == 0),
                            stop=(t == CT - 1),
                        )
                    nc.scalar.activation(
                        q_sb[:, h, s], cp, AF.Identity,
                        bias=bfold["q"][:, h : h + 1], scale=1.0,
                    )
            # vT[n, c] (v bias is applied after attention: softmax rows sum
            # to 1, so attn(v + b) = attn(v) + b)
            vT = pe_.tile([P, 32, C], dt.float32r, tag="vT")
            for jt in range(32):
                vp = mmp.tile([P, C], dt.float32, tag="mm")
                for t in range(CT):
                    nc.tensor.matmul(
                        vp,
                        x_r[:, t, jt * P : (jt + 1) * P],
                        wsc["v"][:, t, :],
                        start=(t == 0),
                        stop=(t == CT - 1),
                    )
                nc.vector.tensor_copy(vT[:, jt], vp)

            # ---------- attention + proj, per 512-wide query chunk ----------
            # The finalize (softmax normalization) and proj for chunk ic-1
            # are emitted after chunk ic's j-loop so their cross-engine
            # latency hides under the next chunk's matmul stream.
            attn = pe_.tile([P, CT, NQ], dt.float32r, tag="attn")
            NIC = NQ // 512
            pend = {}

            def fin_a(ic):
                isl, a_ps, z_ps = pend[ic]
                # 1/Z: reciprocal of psum row 0, broadcast on idle GpSimd
                zr = tmp.tile([1, 512], dt.float32, tag="zr", name=f"zr{ic}")
                nc.vector.reciprocal(zr, z_ps[0:1, :])
                zb = tmp.tile([P, 512], dt.float32, tag="zb", name=f"zb{ic}")
                nc.gpsimd.partition_broadcast(zb, zr)
                pend[ic] = (isl, a_ps, zb)

            def fin_b(ic):
                isl, a_ps, zb = pend.pop(ic)
                for ch in range(CT):
                    nc.vector.tensor_mul(attn[:, ch, isl], a_ps[ch], zb)
                    nc.vector.tensor_scalar_add(
                        attn[:, ch, isl], attn[:, ch, isl],
                        bfold["v"][:, ch : ch + 1],
                    )
                o_sb = tmp.tile([P, CT, 512], dt.float32, tag="o", name=f"o{ic}")
                for h in range(CT):
                    pp = mmp.tile([P, 512], dt.float32, tag="mm")
                    for t in range(CT):
                        nc.tensor.matmul(
                            pp,
                            wp_r[:, t, h * P : (h + 1) * P],
                            attn[:, t, isl],
                            start=(t == 0),
                            stop=(t == CT - 1),
                        )
                    nc.vector.tensor_scalar_add(
                        o_sb[:, h], pp, bvec["p"][:, h : h + 1]
                    )
                    nc.vector.tensor_add(o_sb[:, h], o_sb[:, h], x_r[:, h, isl])
                nc.sync.dma_start(out_ap[:, :, isl], o_sb)

            for ic in range(NIC):
                isl = slice(ic * 512, (ic + 1) * 512)
                a_ps = [accp.tile([P, 512], dt.float32, tag="acc", name=f"acc{ic}_{i}") for i in range(CT)]
                z_ps = zpp.tile([P, 512], dt.float32, tag="z")
                for jt in range(32):
                    st = mmp.tile([P, 512], dt.float32, tag="mm")
                    for h in range(CT):
                        nc.tensor.matmul(
                            st,
                            k_sb[:, h, jt * P : (jt + 1) * P],
                            q_sb[:, h, isl],
                            start=(h == 0),
                            stop=(h == CT - 1),
                        )
                    pt = ptp.tile([P, 512], dt.float32r, tag="pt")
                    nc.scalar.activation(pt, st, AF.Exp, scale=SCALE)
                    for ch in range(CT):
                        nc.tensor.matmul(
                            a_ps[ch],
                            vT[:, jt, ch * P : (ch + 1) * P],
                            pt,
                            start=(jt == 0),
                            stop=(jt == 31),
                        )
                    nc.tensor.matmul(
                        z_ps, e0_sb, pt, start=(jt == 0), stop=(jt == 31)
                    )
                pend[ic] = (isl, a_ps, z_ps)
                fin_a(ic)
                if ic > 0:
                    fin_b(ic - 1)
            fin_b(NIC - 1)

    nc.compile()
    return nc


def _get_nc():
    if "nc" not in _CACHED:
        _CACHED["nc"] = _build()
    return _CACHED["nc"]


def _host_constants():
    sel = np.zeros((P, 4), np.float32)
    e4 = np.zeros((4, P), np.float32)
    for g in range(4):
        sel[g * 32 : (g + 1) * 32, g] = 1.0 / 32.0
        e4[g, g * 32 : (g + 1) * 32] = 1.0
    e0 = np.zeros((P, P), np.float32)
    e0[:, 0] = 1.0  # lhsT col 0 = ones -> psum row 0 = column sums
    return sel, e4, e0


def kernel(x, gn_scale, gn_bias, wq, bq, wk, bk, wv, bv, wp, bp, _trace=False):
    from concourse.bass_utils import run_bass_kernel_spmd

    nc = _get_nc()
    x = np.ascontiguousarray(np.asarray(x, np.float32)).reshape(B, C, N)
    sel, e4, e0 = _host_constants()
    shared = {
        "wqT": np.ascontiguousarray(np.asarray(wq, np.float32).T),
        "wkT": np.ascontiguousarray(np.asarray(wk, np.float32).T),
        "wvT": np.ascontiguousarray(np.asarray(wv, np.float32).T),
        "wpT": np.ascontiguousarray(np.asarray(wp, np.float32).T),
        "bq": np.asarray(bq, np.float32),
        "bk": np.asarray(bk, np.float32),
        "bv": np.asarray(bv, np.float32),
        "bp": np.asarray(bp, np.float32),
        "gn_scale": np.asarray(gn_scale, np.float32),
        "gn_bias": np.asarray(gn_bias, np.float32),
        "sel": sel, "E4": e4, "e0_ones": e0,
    }
    in_maps = []
    for core in range(8):
        b, qh = core // 2, core % 2
        xl = x[b] if qh == 0 else np.ascontiguousarray(
            np.concatenate([x[b][:, NQ:], x[b][:, :NQ]], axis=1)
        )
        in_maps.append({**shared, "x": xl})

    res = run_bass_kernel_spmd(nc, in_maps, core_ids=list(range(8)), trace=_trace)
    out = np.empty((B, C, N), np.float32)
    for core in range(8):
        b, qh = core // 2, core % 2
        out[b][:, qh * NQ : (qh + 1) * NQ] = res.results[core]["out"]
    if _trace:
        _CACHED["last_results"] = res
    return out.reshape(B, C, H, W)


# revision 8
# speedup vs baseline: 1.2164x; 1.0619x over previous
"""AttnBlock (GroupNorm + single-head spatial self-attention + residual) on
8 Trainium2 NeuronCores.

Sharding: batch (4) x query-half (2) -> 8 independent shards, one per core.
Every core runs the SAME program on different data: the host rolls the
flattened spatial axis by 2048 for odd cores so each core's queries are the
first 2048 columns of its local x, while K/V/GroupNorm see the full 4096.

Per-core pipeline (all on device):
  1. GroupNorm stats: bn_stats/bn_aggr per channel, then two tiny fp32
     matmuls reduce across partitions (group stats) and broadcast back.
  2. GN affine (alpha, beta) folded into the Q/K/V weights and biases.
  3. Q/K 1x1 convs -> [c, n] layout; V conv emitted transposed [n, c]
     directly by swapping matmul operands.
  4. Attention with transposed scores: ST[j, i] = k^T q, P = exp(ST/16)
     (softmax max-subtraction skipped; scores are O(10) so exp is safe),
     attn[c, i] = sum_j vT[j, c] P[j, i] accumulated over j in PSUM.
     Softmax denominator Z via a zero-padded ones-column matmul; 1/Z is
     broadcast across partitions with a row-0-ones matmul.
  5. Proj conv + bias + residual, DMA out.

Heavy matmuls run in float32r (full PE rate, ~1.5e-4 rel err); tiny
GroupNorm matmuls in float32.
"""
import numpy as np

B, C, H, W = 4, 256, 64, 64
N = H * W            # 4096 spatial positions
NQ = N // 2          # 2048 queries per core
P = 128              # partitions
CT = C // P          # 2 channel tiles
NUM_GROUPS = 8
EPS = 1e-5
SCALE = float(C) ** -0.5

_CACHED = {}


def _build():
    import concourse.bass as bass
    import concourse.mybir as mybir
    import concourse.tile as tile
    from concourse import bacc

    dt = mybir.dt
    AF = mybir.ActivationFunctionType
    Alu = mybir.AluOpType

    nc = bacc.Bacc("TRN2", debug=False, num_devices=8)

    x_d = nc.dram_tensor("x", [C, N], dt.float32r, kind="ExternalInput")
    wq_d = nc.dram_tensor("wqT", [C, C], dt.float32, kind="ExternalInput")
    wk_d = nc.dram_tensor("wkT", [C, C], dt.float32, kind="ExternalInput")
    wv_d = nc.dram_tensor("wvT", [C, C], dt.float32, kind="ExternalInput")
    wp_d = nc.dram_tensor("wpT", [C, C], dt.float32, kind="ExternalInput")
    bq_d = nc.dram_tensor("bq", [C], dt.float32, kind="ExternalInput")
    bk_d = nc.dram_tensor("bk", [C], dt.float32, kind="ExternalInput")
    bv_d = nc.dram_tensor("bv", [C], dt.float32, kind="ExternalInput")
    bp_d = nc.dram_tensor("bp", [C], dt.float32, kind="ExternalInput")
    gsc_d = nc.dram_tensor("gn_scale", [C], dt.float32, kind="ExternalInput")
    gbi_d = nc.dram_tensor("gn_bias", [C], dt.float32, kind="ExternalInput")
    sel_d = nc.dram_tensor("sel", [P, 4], dt.float32, kind="ExternalInput")
    e4_d = nc.dram_tensor("E4", [4, P], dt.float32, kind="ExternalInput")
    e0_d = nc.dram_tensor("e0_ones", [P, P], dt.float32r, kind="ExternalInput")
    out_d = nc.dram_tensor("out", [C, NQ], dt.float32, kind="ExternalOutput")

    x_ap = x_d.ap().rearrange("(t p) n -> p t n", p=P)
    out_ap = out_d.ap().rearrange("(t p) n -> p t n", p=P)

    with tile.TileContext(nc) as tc:
        with (
            nc.allow_low_precision(reason="float32r rounding is intentional"),
            tc.tile_pool(name="persist", bufs=1) as pe_,
            tc.tile_pool(name="pt", bufs=5) as ptp,
            tc.tile_pool(name="tmp", bufs=3) as tmp,
            tc.tile_pool(name="mm", bufs=3, space="PSUM") as mmp,
            tc.tile_pool(name="acc", bufs=4, space="PSUM") as accp,
            tc.tile_pool(name="zp", bufs=1, space="PSUM") as zpp,
        ):
            # ---------- load persistent data ----------
            x_r = pe_.tile([P, CT, N], dt.float32r, tag="x")
            stats = pe_.tile([P, CT, 8, 6], dt.float32, tag="stats")
            for ck in range(8):
                s = slice(ck * 512, (ck + 1) * 512)
                nc.sync.dma_start(x_r[:, :, s], x_ap[:, :, s])
                for t in range(CT):
                    nc.vector.bn_stats(stats[:, t, ck, :], x_r[:, t, s])

            wT = {}
            for nm, d in (("q", wq_d), ("k", wk_d), ("v", wv_d), ("p", wp_d)):
                wT[nm] = pe_.tile([P, CT, C], dt.float32, tag=f"w{nm}", name=f"w{nm}")
                nc.sync.dma_start(wT[nm], d.ap().rearrange("(t p) o -> p t o", p=P))
            bvec = {}
            for nm, d in (("q", bq_d), ("k", bk_d), ("v", bv_d), ("p", bp_d),
                          ("gsc", gsc_d), ("gbi", gbi_d)):
                bvec[nm] = pe_.tile([P, CT], dt.float32, tag=f"b{nm}", name=f"b{nm}")
                nc.sync.dma_start(bvec[nm], d.ap().rearrange("(t p) -> p t", p=P))
            sel_sb = pe_.tile([P, 4], dt.float32, tag="sel")
            nc.sync.dma_start(sel_sb, sel_d.ap())
            e4_sb = pe_.tile([4, P], dt.float32, tag="e4")
            nc.sync.dma_start(e4_sb, e4_d.ap())
            e0_sb = pe_.tile([P, P], dt.float32r, tag="e0")
            nc.sync.dma_start(e0_sb, e0_d.ap())
            zeros4 = pe_.tile([P, 4], dt.float32, tag="zeros4")
            nc.vector.memset(zeros4, 0.0)
            # ---------- GroupNorm statistics ----------
            mv = pe_.tile([P, CT, 2], dt.float32, tag="mv")
            for t in range(CT):
                nc.vector.bn_aggr(mv[:, t, :], stats[:, t])
            # stats_cat cols: mean_t0, mean_t1, meansq_t0, meansq_t1
            scat = pe_.tile([P, 4], dt.float32, tag="scat")
            for t in range(CT):
                nc.vector.tensor_copy(scat[:, t : t + 1], mv[:, t, 0:1])
                sq = tmp.tile([P, 1], dt.float32, tag="sq")
                nc.vector.tensor_mul(sq, mv[:, t, 0:1], mv[:, t, 0:1])
                nc.vector.tensor_add(scat[:, 2 + t : 3 + t], sq, mv[:, t, 1:2])
            gs_ps = mmp.tile([4, 4], dt.float32, tag="mm")
            # dummy zero-contribution matmul: boots the PE pipeline early
            # (absorbs first-instruction latency) while stats still stream
            nc.tensor.matmul(gs_ps, zeros4, sel_sb[:, 0:4], start=True, stop=False)
            nc.tensor.matmul(gs_ps, sel_sb, scat, start=False, stop=True)
            gs = pe_.tile([4, 4], dt.float32, tag="gs")
            nc.vector.tensor_copy(gs, gs_ps)
            # var = meansq - mean^2 ; rstd = rsqrt(var + eps) + one Newton step
            msq = pe_.tile([4, 2], dt.float32, tag="msq")
            nc.vector.tensor_mul(msq, gs[:, 0:2], gs[:, 0:2])
            veps = pe_.tile([4, 2], dt.float32, tag="veps")
            nc.vector.tensor_sub(veps, gs[:, 2:4], msq)
            nc.vector.tensor_scalar_add(veps, veps, EPS)
            sqv = pe_.tile([4, 2], dt.float32, tag="sqv")
            nc.scalar.activation(sqv, veps, AF.Sqrt)
            y0 = pe_.tile([4, 2], dt.float32, tag="y0")
            nc.vector.reciprocal(y0, sqv)
            yy = pe_.tile([4, 2], dt.float32, tag="yy")
            nc.vector.tensor_mul(yy, y0, y0)
            nc.vector.tensor_mul(yy, veps, yy)
            nc.vector.tensor_scalar(yy, yy, -0.5, 1.5, Alu.mult, Alu.add)
            mr = pe_.tile([4, 4], dt.float32, tag="mr")
            nc.vector.tensor_copy(mr[:, 0:2], gs[:, 0:2])
            nc.vector.tensor_mul(mr[:, 2:4], y0, yy)
            bc_ps = mmp.tile([P, 4], dt.float32, tag="mm")
            nc.tensor.matmul(bc_ps, e4_sb, mr, start=True, stop=True)
            bc = pe_.tile([P, 4], dt.float32, tag="bc")
            nc.vector.tensor_copy(bc, bc_ps)
            alpha = pe_.tile([P, CT], dt.float32, tag="alpha")
            nc.vector.tensor_mul(alpha, bc[:, 2:4], bvec["gsc"])
            beta = pe_.tile([P, CT], dt.float32, tag="beta")
            nc.vector.tensor_mul(beta, bc[:, 0:2], alpha)
            nc.vector.tensor_sub(beta, bvec["gbi"], beta)

            # ---------- fold GN affine into weights & biases ----------
            wsc = {}
            for nm in ("q", "k", "v"):
                wsc[nm] = pe_.tile([P, CT, C], dt.float32r, tag=f"wsc{nm}", name=f"wsc{nm}")
                for t in range(CT):
                    nc.vector.tensor_scalar_mul(
                        wsc[nm][:, t], wT[nm][:, t], alpha[:, t : t + 1]
                    )
            wp_r = pe_.tile([P, CT, C], dt.float32r, tag="wscp")
            nc.vector.tensor_copy(wp_r, wT["p"])

            bfold = {}
            for nm in ("q", "k", "v"):
                bfold[nm] = pe_.tile([P, CT], dt.float32, tag=f"bf{nm}", name=f"bf{nm}")
                for h in range(CT):
                    bb_ps = mmp.tile([P, 1], dt.float32, tag="mm")
                    for t in range(CT):
                        nc.tensor.matmul(
                            bb_ps,
                            wT[nm][:, t, h * P : (h + 1) * P],
                            beta[:, t : t + 1],
                            start=(t == 0),
                            stop=(t == CT - 1),
                        )
                    nc.vector.tensor_add(
                        bfold[nm][:, h : h + 1], bb_ps, bvec[nm][:, h : h + 1]
                    )

            # proj bias absorbs the v-bias: wp @ (attn + bv') + bp
            #   = wp @ attn + (bp + wp @ bv')
            bpp = pe_.tile([P, CT], dt.float32, tag="bpp")
            for h in range(CT):
                bb2 = mmp.tile([P, 1], dt.float32, tag="mm")
                for t in range(CT):
                    nc.tensor.matmul(
                        bb2,
                        wT["p"][:, t, h * P : (h + 1) * P],
                        bfold["v"][:, t : t + 1],
                        start=(t == 0),
                        stop=(t == CT - 1),
                    )
                nc.vector.tensor_add(
                    bpp[:, h : h + 1], bb2, bvec["p"][:, h : h + 1]
                )

            # ---------- Q/K/V 1x1 convs ----------
            k_sb = pe_.tile([P, CT, N], dt.float32r, tag="k")
            q_sb = pe_.tile([P, CT, NQ], dt.float32r, tag="q")
            for h in range(CT):
                for ck in range(8):
                    s = slice(ck * 512, (ck + 1) * 512)
                    cp = mmp.tile([P, 512], dt.float32, tag="mm")
                    for t in range(CT):
                        nc.tensor.matmul(
                            cp,
                            wsc["k"][:, t, h * P : (h + 1) * P],
                            x_r[:, t, s],
                            start=(t == 0),
                            stop=(t == CT - 1),
                        )
                    nc.scalar.activation(
                        k_sb[:, h, s], cp, AF.Identity,
                        bias=bfold["k"][:, h : h + 1], scale=1.0,
                    )
            for h in range(CT):
                for ck in range(4):
                    s = slice(ck * 512, (ck + 1) * 512)
                    cp = mmp.tile([P, 512], dt.float32, tag="mm")
                    for t in range(CT):
                        nc.tensor.matmul(
                            cp,
                            wsc["q"][:, t, h * P : (h + 1) * P],
                            x_r[:, t, s],
                            start=(t == 0),
                            stop=(t == CT - 1),
                        )
                    nc.scalar.activation(
                        q_sb[:, h, s], cp, AF.Identity,
                        bias=bfold["q"][:, h : h + 1], scale=1.0,
                    )
            # vT[n, c] (v bias is applied after attention: softmax rows sum
            # to 1, so attn(v + b) = attn(v) + b)
            vT = pe_.tile([P, 32, C], dt.float32r, tag="vT")
            for jt in range(32):
                vp = mmp.tile([P, C], dt.float32, tag="mm")
                for t in range(CT):
                    nc.tensor.matmul(
                        vp,
                        x_r[:, t, jt * P : (jt + 1) * P],
                        wsc["v"][:, t, :],
                        start=(t == 0),
                        stop=(t == CT - 1),
                    )
                nc.vector.tensor_copy(vT[:, jt], vp)

            # ---------- attention + proj, per 512-wide query chunk ----------
            # The finalize (softmax normalization) and proj for chunk ic-1
            # are emitted after chunk ic's j-loop so their cross-engine
            # latency hides under the next chunk's matmul stream.
            attn = pe_.tile([P, CT, NQ], dt.float32r, tag="attn")
            NIC = NQ // 512
            pend = {}

            def fin_a(ic):
                isl, a_ps, z_ps = pend[ic]
                # copy Z row out of PSUM first (frees the z bank for the
                # next chunk), then 1/Z + broadcast off the critical path
                zc = tmp.tile([1, 3, 512], dt.float32, tag="zc", name=f"zc{ic}")
                nc.vector.tensor_copy(zc[:, 0, :], z_ps[0:1, :])
                nc.vector.reciprocal_approx_accurate(
                    zc[:, 1, :], zc[:, 0, :], zc[:, 2, :]
                )
                zb = tmp.tile([P, 512], dt.float32, tag="zb", name=f"zb{ic}")
                nc.gpsimd.partition_broadcast(zb, zc[:, 1, :])
                pend[ic] = (isl, a_ps, zb)

            def fin_b(ic):
                isl, a_ps, zb = pend.pop(ic)
                for ch in range(CT):
                    nc.vector.tensor_mul(attn[:, ch, isl], a_ps[ch], zb)
                o_sb = tmp.tile([P, CT, 512], dt.float32, tag="o", name=f"o{ic}")
                for h in range(CT):
                    pp = mmp.tile([P, 512], dt.float32, tag="mm")
                    for t in range(CT):
                        nc.tensor.matmul(
                            pp,
                            wp_r[:, t, h * P : (h + 1) * P],
                            attn[:, t, isl],
                            start=(t == 0),
                            stop=(t == CT - 1),
                        )
                    nc.vector.tensor_scalar_add(
                        o_sb[:, h], pp, bpp[:, h : h + 1]
                    )
                    nc.vector.tensor_add(o_sb[:, h], o_sb[:, h], x_r[:, h, isl])
                nc.sync.dma_start(out_ap[:, :, isl], o_sb)

            for ic in range(NIC):
                isl = slice(ic * 512, (ic + 1) * 512)
                a_ps = [accp.tile([P, 512], dt.float32, tag="acc", name=f"acc{ic}_{i}") for i in range(CT)]
                z_ps = zpp.tile([P, 512], dt.float32, tag="z")
                for jt in range(32):
                    st = mmp.tile([P, 512], dt.float32, tag="mm")
                    for h in range(CT):
                        nc.tensor.matmul(
                            st,
                            k_sb[:, h, jt * P : (jt + 1) * P],
                            q_sb[:, h, isl],
                            start=(h == 0),
                            stop=(h == CT - 1),
                        )
                    pt = ptp.tile([P, 512], dt.float32r, tag="pt")
                    nc.scalar.activation(pt, st, AF.Exp, scale=SCALE)
                    for ch in range(CT):
                        nc.tensor.matmul(
                            a_ps[ch],
                            vT[:, jt, ch * P : (ch + 1) * P],
                            pt,
                            start=(jt == 0),
                            stop=(jt == 31),
                        )
                    nc.tensor.matmul(
                        z_ps, e0_sb, pt, start=(jt == 0), stop=(jt == 31)
                    )
                pend[ic] = (isl, a_ps, z_ps)
                fin_a(ic)
                if ic > 0:
                    fin_b(ic - 1)
            fin_b(NIC - 1)

    nc.compile()
    return nc


def _get_nc():
    if "nc" not in _CACHED:
        _CACHED["nc"] = _build()
    return _CACHED["nc"]


def _host_constants():
    sel = np.zeros((P, 4), np.float32)
    e4 = np.zeros((4, P), np.float32)
    for g in range(4):
        sel[g * 32 : (g + 1) * 32, g] = 1.0 / 32.0
        e4[g, g * 32 : (g + 1) * 32] = 1.0
    e0 = np.zeros((P, P), np.float32)
    e0[:, 0] = 1.0  # lhsT col 0 = ones -> psum row 0 = column sums
    return sel, e4, e0


def kernel(x, gn_scale, gn_bias, wq, bq, wk, bk, wv, bv, wp, bp, _trace=False):
    from concourse.bass_utils import run_bass_kernel_spmd

    nc = _get_nc()
    x = np.ascontiguousarray(np.asarray(x, np.float32)).reshape(B, C, N)
    sel, e4, e0 = _host_constants()
    shared = {
        "wqT": np.ascontiguousarray(np.asarray(wq, np.float32).T),
        "wkT": np.ascontiguousarray(np.asarray(wk, np.float32).T),
        "wvT": np.ascontiguousarray(np.asarray(wv, np.float32).T),
        "wpT": np.ascontiguousarray(np.asarray(wp, np.float32).T),
        "bq": np.asarray(bq, np.float32),
        "bk": np.asarray(bk, np.float32),
        "bv": np.asarray(bv, np.float32),
        "bp": np.asarray(bp, np.float32),
        "gn_scale": np.asarray(gn_scale, np.float32),
        "gn_bias": np.asarray(gn_bias, np.float32),
        "sel": sel, "E4": e4, "e0_ones": e0,
    }
    in_maps = []
    for core in range(8):
        b, qh = core // 2, core % 2
        xl = x[b] if qh == 0 else np.ascontiguousarray(
            np.concatenate([x[b][:, NQ:], x[b][:, :NQ]], axis=1)
        )
        in_maps.append({**shared, "x": xl})

    res = run_bass_kernel_spmd(nc, in_maps, core_ids=list(range(8)), trace=_trace)
    out = np.empty((B, C, N), np.float32)
    for core in range(8):
        b, qh = core // 2, core % 2
        out[b][:, qh * NQ : (qh + 1) * NQ] = res.results[core]["out"]
    if _trace:
        _CACHED["last_results"] = res
    return out.reshape(B, C, H, W)
